# revision 2
# baseline (speedup 1.0000x reference)
"""GCN encoder (2-layer GCNConv + relu, concat) on 8 Trainium2 NeuronCores.

Strategy (graph/data parallel, per sharding hint):
  - Nodes partitioned across 8 cores (12500 each); each core owns the edges
    whose dst lands in its partition. Self-loops are appended as regular
    edges on the host (same as the reference's PyG-style GCNConv).
  - "Halo exchange" is realized by replicating the (dinv-scaled) feature
    matrix to every core per layer: each core computes the full
    table = (dinv * x) @ W on-device (the weight matrices are replicated),
    then gathers the rows for its own edges with dma_gather and
    segment-sums them into its dst chunks via one-hot matmuls on the PE.
  - Between the two layers the per-core h1 shards are gathered on the host
    and redistributed (the two launches run the SAME compiled program).

Math: out[d] = relu( dinv[d] * sum_{e: dst=d} (dinv[src_e] * (x @ W)[src_e]) + b )
with deg counted on dst (incl. self-loop), matching jax reference exactly.
"""

import numpy as np
from contextlib import ExitStack

P = 128
N_NODES = 100_000
N_CORES = 8
PER_CORE = N_NODES // N_CORES          # 12500
N_CHUNK = (PER_CORE + P - 1) // P      # 98 chunks of 128 dsts (last has 84)
OWN_PAD = N_CHUNK * P                  # 12544
N_PAD = 100_096                        # table rows, divisible by 4
QROWS = N_PAD // 4                     # 25024 rows per src quadrant (< int16 max)
SC_SIZES = [4] * 24 + [2]              # superchunks: 24x4 + 1x2 = 98 chunks
HID = 128

_program_cache = {}


def _build_program(T):
    """Bass program for one GCN layer, parametrized by tiles-per-bucket T."""
    from concourse import bass, mybir, bacc
    from concourse import library_config
    import concourse.tile as tile

    f16 = mybir.dt.float16
    f32 = mybir.dt.float32
    i16 = mybir.dt.int16

    TOT_IDX = N_CHUNK * 4 * T * P          # gather slots per core
    NDR = N_CHUNK * 4 * T                  # dst_rel columns per core

    nc = bacc.Bacc(target_bir_lowering=False)
    xT = nc.declare_dram_parameter("xT", [P, N_PAD], f16, isOutput=False)
    W = nc.declare_dram_parameter("W", [P, HID], f16, isOutput=False)
    bmat = nc.declare_dram_parameter("bmat", [P, HID], f32, isOutput=False)
    idxs = nc.declare_dram_parameter("idxs", [P, TOT_IDX // 16], i16, isOutput=False)
    dstrel = nc.declare_dram_parameter("dstrel", [P, NDR], f16, isOutput=False)
    dinv = nc.declare_dram_parameter("dinv", [P, N_CHUNK], f32, isOutput=False)
    hout = nc.declare_dram_parameter("hout", [N_CHUNK, P, HID], f32, isOutput=True)

    table = nc.dram_tensor("table", [N_PAD, HID], f16)

    # ---- Phase 1: table = xT^T @ W  (full table, replicated on every core)
    with tile.TileContext(nc) as tc:
        with ExitStack() as ctx:
            cpool = ctx.enter_context(tc.tile_pool(name="c1", bufs=1))
            wt = cpool.tile([P, HID], f16)
            nc.sync.dma_start(out=wt[:], in_=W[:, :])
            xpool = ctx.enter_context(tc.tile_pool(name="xs", bufs=3))
            ppool = ctx.enter_context(tc.tile_pool(name="ps1", bufs=4, space="PSUM"))
            spool = ctx.enter_context(tc.tile_pool(name="st1", bufs=4))
            SLAB = 4096                    # nodes per input DMA (24x4096 + 1792)
            for a in range(0, N_PAD, SLAB):
                n = min(SLAB, N_PAD - a)
                xs = xpool.tile([P, n], f16, tag="xs")
                nc.sync.dma_start(out=xs[:], in_=xT[:, a:a + n])
                for g in range(0, n, 512):
                    gn = min(512, n - g)
                    psum4 = ppool.tile([P, 512 // P, P], f32, space="PSUM", tag="ps")
                    for t in range(0, gn, P):
                        nc.tensor.matmul(
                            out=psum4[:, t // P, :],
                            lhsT=xs[:, g + t:g + t + P],
                            rhs=wt[:],
                            start=True, stop=True,
                        )
                    st = spool.tile([P, 512 // P, P], f16, tag="st")
                    nc.scalar.copy(
                        out=st[:, : gn // P, :],
                        in_=psum4[:, : gn // P, :],
                    )
                    for t in range(0, gn, P):
                        nc.sync.dma_start(
                            out=table[a + g + t: a + g + t + P, :],
                            in_=st[:, t // P, :],
                        )

    # ---- Phase 2: gather + segment-sum + epilogue
    with tile.TileContext(nc) as tc:
        with ExitStack() as ctx:
            nc.gpsimd.load_library(library_config.mlp)
            cpool = ctx.enter_context(tc.tile_pool(name="c2", bufs=1))
            iota16 = cpool.tile([P, P], i16)
            nc.gpsimd.iota(iota16[:], pattern=[[1, P]], base=0, channel_multiplier=0)
            iotaf = cpool.tile([P, P], f16)
            nc.vector.tensor_copy(out=iotaf[:], in_=iota16[:])
            drl = cpool.tile([P, NDR], f16)
            nc.sync.dma_start(out=drl[:], in_=dstrel[:, :])
            dv = cpool.tile([P, N_CHUNK], f32)
            nc.sync.dma_start(out=dv[:], in_=dinv[:, :])
            bm = cpool.tile([P, HID], f32)
            nc.sync.dma_start(out=bm[:], in_=bmat[:, :])

            ixpool = ctx.enter_context(tc.tile_pool(name="ix", bufs=2))
            mpool = ctx.enter_context(tc.tile_pool(name="msg", bufs=6))
            spool = ctx.enter_context(tc.tile_pool(name="S", bufs=3))
            ppool = ctx.enter_context(tc.tile_pool(name="ps2", bufs=4, space="PSUM"))
            hpool = ctx.enter_context(tc.tile_pool(name="h", bufs=4))

            idx_off = 0                    # column offset into idxs (16-wrapped)
            k0 = 0                         # global chunk counter
            for csc in SC_SIZES:
                NI = csc * T * P           # idxs per gather
                w16 = NI // 16             # wrapped cols per gather
                ixt = ixpool.tile([P, 4 * w16], i16, tag="ix")
                nc.sync.dma_start(out=ixt[:], in_=idxs[:, idx_off:idx_off + 4 * w16])
                msgs = []
                for q in range(4):
                    m = mpool.tile([P, csc * T, P], f16, tag="msg")
                    nc.gpsimd.dma_gather(
                        m[:, :, :],
                        table[QROWS * q: QROWS * (q + 1), :],
                        ixt[:, q * w16:(q + 1) * w16],
                        NI, NI, P,
                        single_packet=False,
                    )
                    msgs.append(m)
                for ci in range(csc):
                    k = k0 + ci
                    S = spool.tile([P, 4 * T, P], f16, tag="S")
                    nc.vector.tensor_tensor(
                        out=S[:, :, :],
                        in0=drl[:, k * 4 * T:(k + 1) * 4 * T, None].to_broadcast([P, 4 * T, P]),
                        in1=iotaf[:, None, :].to_broadcast([P, 4 * T, P]),
                        op=mybir.AluOpType.is_equal,
                    )
                    psum = ppool.tile([P, HID], f32, space="PSUM", tag="ps")
                    nmm = 4 * T
                    mm = 0
                    for q in range(4):
                        for t in range(T):
                            nc.tensor.matmul(
                                out=psum[:],
                                lhsT=S[:, q * T + t, :],
                                rhs=msgs[q][:, ci * T + t, :],
                                start=(mm == 0), stop=(mm == nmm - 1),
                            )
                            mm += 1
                    t2 = hpool.tile([P, HID], f32, tag="t2")
                    nc.vector.scalar_tensor_tensor(
                        out=t2[:], in0=psum[:], scalar=dv[:, k:k + 1], in1=bm[:],
                        op0=mybir.AluOpType.mult, op1=mybir.AluOpType.add,
                    )
                    h = hpool.tile([P, HID], f32, tag="h")
                    nc.vector.tensor_scalar_max(h[:], t2[:], 0.0)
                    nc.sync.dma_start(out=hout[k, :, :], in_=h[:])
                idx_off += 4 * w16
                k0 += csc

    nc.finalize()
    return nc


def _prep_edges(src, dst):
    """Bucket edges by (core, chunk, quadrant); build per-core gather/S inputs."""
    src = np.concatenate([src, np.arange(N_NODES, dtype=np.int64)])   # self loops
    dst = np.concatenate([dst, np.arange(N_NODES, dtype=np.int64)])
    core = dst // PER_CORE
    dloc = dst % PER_CORE
    chunk = dloc // P
    drel = dloc % P
    quad = src // QROWS
    sloc = (src - quad * QROWS).astype(np.int64)

    bid = ((core * N_CHUNK + chunk) * 4 + quad).astype(np.int64)
    nb = N_CORES * N_CHUNK * 4
    counts = np.bincount(bid, minlength=nb)
    T = int(np.ceil(counts.max() / P))

    order = np.argsort(bid, kind="stable")
    bid_s = bid[order]
    sloc_s = sloc[order]
    drel_s = drel[order]
    starts = np.zeros(nb + 1, np.int64)
    np.cumsum(counts, out=starts[1:])
    rank = np.arange(len(bid_s)) - starts[bid_s]          # slot within bucket

    # global slot of each edge inside its core's flat gather array
    cr = bid_s // (N_CHUNK * 4)
    ch = (bid_s // 4) % N_CHUNK
    qd = bid_s % 4
    sc = np.minimum(ch // 4, 24)
    ci = ch - np.where(sc < 24, sc * 4, 96)
    csc = np.where(sc < 24, 4, 2)
    sc_base = np.where(sc < 24, sc * 4 * 4 * T * P, 96 * 4 * T * P)   # slots before this sc
    slot = sc_base + qd * csc * T * P + ci * T * P + rank

    TOT = N_CHUNK * 4 * T * P
    NDR = N_CHUNK * 4 * T
    idx_flat = np.zeros((N_CORES, TOT), np.int16)
    drel_col = np.full((N_CORES, P, NDR), 300.0, np.float16)
    idx_flat[cr, slot] = sloc_s.astype(np.int16)
    # dst_rel position: column = chunk*4T + quad*T + t, partition = slot%128
    t_in = rank // P
    col = ch * 4 * T + qd * T + t_in
    drel_col[cr, (rank % P), col] = drel_s.astype(np.float16)

    # wrap idx per core: i -> [i%16, i//16], replicated to 128 partitions
    idx_wrapped = np.empty((N_CORES, P, TOT // 16), np.int16)
    for c in range(N_CORES):
        w = idx_flat[c].reshape(-1, 16).T                 # [16, TOT/16]
        idx_wrapped[c] = np.tile(w, (8, 1))
    return T, idx_wrapped, drel_col


def _run_layer(nc, run_fn, xs_scaled, Wl, bl, idx_wrapped, drel_col, dinv_arr):
    """One launch: returns h [N_NODES, HID] fp32."""
    xpad = np.zeros((N_PAD, HID), np.float32)
    xpad[:N_NODES] = xs_scaled
    xT = np.ascontiguousarray(xpad.T).astype(np.float16)
    Wh = Wl.astype(np.float16)
    bm = np.tile(bl.astype(np.float32)[None, :], (P, 1))

    dpad = np.zeros(N_CORES * OWN_PAD, np.float32)
    dv = dinv_arr.reshape(N_CORES, PER_CORE)
    for c in range(N_CORES):
        dpad[c * OWN_PAD: c * OWN_PAD + PER_CORE] = dv[c]
    dvt = dpad.reshape(N_CORES, N_CHUNK, P).transpose(0, 2, 1)  # [cores, 128, 98]

    in_maps = []
    for c in range(N_CORES):
        in_maps.append({
            "xT": xT, "W": Wh, "bmat": bm,
            "idxs": idx_wrapped[c],
            "dstrel": drel_col[c],
            "dinv": np.ascontiguousarray(dvt[c]),
        })
    res = run_fn(nc, in_maps, list(range(N_CORES)))
    h = np.empty((N_NODES, HID), np.float32)
    for c in range(N_CORES):
        hc = res.results[c]["hout"].reshape(OWN_PAD, HID)
        h[c * PER_CORE:(c + 1) * PER_CORE] = hc[:PER_CORE]
    return h


def kernel(x, edge_index, W1, b1, W2, b2):
    from concourse.bass_utils import run_bass_kernel_spmd

    x = np.asarray(x, dtype=np.float32)
    edge_index = np.asarray(edge_index)
    W1 = np.asarray(W1, np.float32); b1 = np.asarray(b1, np.float32)
    W2 = np.asarray(W2, np.float32); b2 = np.asarray(b2, np.float32)
    src = edge_index[0].astype(np.int64)
    dst = edge_index[1].astype(np.int64)

    deg = np.bincount(dst, minlength=N_NODES).astype(np.float64) + 1.0
    dinv = (1.0 / np.sqrt(deg)).astype(np.float32)

    T, idx_wrapped, drel_col = _prep_edges(src, dst)

    if T not in _program_cache:
        _program_cache[T] = _build_program(T)
    nc = _program_cache[T]

    h1 = _run_layer(nc, run_bass_kernel_spmd, x * dinv[:, None], W1, b1,
                    idx_wrapped, drel_col, dinv)
    h2 = _run_layer(nc, run_bass_kernel_spmd, h1 * dinv[:, None], W2, b2,
                    idx_wrapped, drel_col, dinv)
    return np.concatenate([h1, h2], axis=1).astype(np.float32)
